# revision 12
# baseline (speedup 1.0000x reference)
"""Trainium2 Bass kernel for nn_Attention (LN -> QKV -> alibi attention -> out-proj).

Full shapes: x[2,2048,1024], alibi[1,16,2048,2048], w_qkv[1024,3072], w_out[1024,1024].
Sharding: tensor-parallel over heads. Core c owns heads {2c, 2c+1} for BOTH batches.
Each core computes a partial out-projection; the host sums the 8 partials (the
tensor-parallel reduction) and transposes back.

V1 design (from perfetto analysis of the 341us baseline: PE 275us busy, ACT 188us,
DVE 197us, with exp(scores) = 142us the hard ACT floor):
  - exp(s + a) = exp(s) * exp(a): the host precomputes ea = exp(alibi^T) in bf16,
    tiled [hh, ihalf, jc, 128, 1024] (contiguous per tile). The kernel multiplies
    it into the exp'd scores with a DVE bf16 2x-mode tensor_tensor (594ns/tile)
    instead of injecting alibi into PSUM via PE identity-matmuls + DVE 1x adds.
    Removes 27.6us PE + ~84us DVE vs baseline.
  - scores for the 2 local heads are row-packed on the PE: K=64 matmuls for h0
    (array rows 0-63) and h1 (rows 64-127) execute concurrently (tile_position
    auto-derived from base partitions) into separate PSUM banks. Halves score
    matmul time to 27.6us.
  - LN fold: qkv_psum = W_eff^T x + nwsum x mean + qkvb x std (rank-1 terms fed
    as one extra K=2 matmul from a [2,N] bf16 [mean;std] row-pair), then a single
    DVE multiply by broadcast rstd evicts. Kills the baseline's ACT copy +
    scalar_tensor_tensor + ACT identity chain.
  - v natural layout via DMA crossbar transpose (dma_start transpose=True),
    replacing PE transposes + DVE copies.
  - PSUM: pool ps_sc (2x2 banks, score/exp ping-pong + stats) + ps_acc (2x2
    banks: attn@v accumulators, QKV groups, out-proj groups) = 8 banks.
  - Coarse phase interleave so ACT exp starts early and PE fills attention-phase
    gaps: stats/qkv b0 -> attn b0-ih0 -> qkv-b1 part1 -> attn b0-ih1 -> qkv-b1
    part2 -> attn b1-ih0 -> q-ih1 + outproj-b0 -> attn b1-ih1 -> outproj-b1.
"""

import sys

sys.path.insert(0, "/opt/trn_rl_repo")

from contextlib import ExitStack

import numpy as np
import ml_dtypes

import concourse.bass as bass
from concourse import bacc
import concourse.mybir as mybir
import concourse.tile as tile
from concourse.bass_utils import run_bass_kernel_spmd

F32 = mybir.dt.float32
BF16 = mybir.dt.bfloat16

B, N, D = 2, 2048, 1024
H, DH = 16, 64
NCORES = 8
HL = H // NCORES          # local heads per core = 2
CL = HL * DH              # local head channels = 128
LN_EPS = 1e-5
SCALE = DH ** -0.5
KT = D // 128             # 8 d-tiles
JC = N // 128             # 16 j-chunks

_CACHED_NC = None


def build_nc() -> bass.Bass:
    nc = bacc.Bacc(None)
    xt_d = nc.declare_dram_parameter("xt", [B, D, N], BF16, isOutput=False)
    ea_d = nc.declare_dram_parameter("ea", [HL, 2, JC, 128, 1024], BF16, isOutput=False)
    wqkv_d = nc.declare_dram_parameter("wqkv", [D, 3 * CL], BF16, isOutput=False)
    wqkv2_d = nc.declare_dram_parameter("wqkv2", [2, 3 * CL], BF16, isOutput=False)
    wout_d = nc.declare_dram_parameter("wout", [CL, D], BF16, isOutput=False)
    ones_d = nc.declare_dram_parameter("ones", [128, 1], BF16, isOutput=False)
    out_d = nc.declare_dram_parameter("out", [B, D, N], BF16, isOutput=True)

    with tile.TileContext(nc) as tc, ExitStack() as ctx:
        ep = lambda **kw: ctx.enter_context(tc.tile_pool(**kw))
        cpool = ep(name="const", bufs=1)
        xt_pool = ep(name="xt", bufs=24)     # [128,1024] bf16 tiles (b, kt, ihalf)
        sq_pool = ep(name="sq", bufs=4)
        sm_pool = ep(name="small", bufs=2)
        qk_pool = ep(name="qk", bufs=1)      # per-batch qT/kT, all resident
        vn_pool = ep(name="vn", bufs=1)      # 4 resident tiles (b x head)
        ea_pool = ep(name="ea", bufs=8)
        ate_pool = ep(name="ate", bufs=6)
        at_pool = ep(name="at", bufs=8)
        ao_pool = ep(name="aos", bufs=1)
        ob_pool = ep(name="ob", bufs=3)
        bc_pool = ep(name="bc", bufs=4)
        ms_pool = ep(name="ms", bufs=2)
        rrbc_pool = ep(name="rrbc", bufs=2)
        aor_pool = ep(name="aor", bufs=2)
        dscr_pool = ep(name="dscr", bufs=4, space="DRAM")
        ps_sc = ep(name="ps_sc", bufs=2, space="PSUM")
        ps_acc = ep(name="ps_acc", bufs=2, space="PSUM")

        # ---- constants ----
        zero_sb = cpool.tile([128, 1], F32, name="zero_sb")
        nc.vector.memset(zero_sb, 0.0)
        nc.const_aps.aps[(F32, 0.0)] = zero_sb[:, 0:1]
        eps_sb = cpool.tile([128, 1], F32, name="eps_sb")
        nc.vector.memset(eps_sb, LN_EPS)
        ones_sb = cpool.tile([128, 1], BF16, name="ones_sb")
        nc.sync.dma_start(out=ones_sb, in_=ones_d[:, :])

        # x tiles [128, 1024] per (b, kt, ihalf); batch 0 first
        xts = [[[None] * 2 for _ in range(KT)] for _ in range(B)]

        def load_x(b, kt, ihalf):
            t = xt_pool.tile([128, 1024], BF16, name=f"xt_{b}_{kt}_{ihalf}", tag="xt")
            isl = slice(ihalf * 1024, (ihalf + 1) * 1024)
            nc.sync.dma_start(out=t, in_=xt_d[b, kt * 128:(kt + 1) * 128, isl])
            xts[b][kt][ihalf] = t

        for kt in range(KT):
            for ihalf in range(2):
                load_x(0, kt, ihalf)
        wqkv_sb = cpool.tile([128, KT, 3 * CL], BF16, name="wqkv_sb")
        nc.sync.dma_start(out=wqkv_sb, in_=wqkv_d.rearrange("(t p) c -> p t c", p=128))
        wqkv2_sb = cpool.tile([2, 3 * CL], BF16, name="wqkv2_sb")
        nc.sync.dma_start(out=wqkv2_sb, in_=wqkv2_d[:, :])
        wout_sb = cpool.tile([128, D], BF16, name="wout_sb")
        nc.sync.dma_start(out=wout_sb, in_=wout_d[:, :])
        for kt in range(KT):
            for ihalf in range(2):
                load_x(1, kt, ihalf)

        # per-batch state
        qTs, kTs, vns, aos = [], [], [], []
        bcs_all, ms_all, rstdj_all = [], [], []
        for b in range(B):
            qTs.append(qk_pool.tile([128, N], BF16, name=f"qT_{b}", tag=f"qT{b}"))
            kTs.append(qk_pool.tile([128, N], BF16, name=f"kT_{b}", tag=f"kT{b}"))
            vb = []
            for hh in range(HL):
                vn = vn_pool.tile([128, JC, DH + 1], BF16, name=f"vn_{b}_{hh}", tag=f"vn{b}{hh}")
                nc.gpsimd.memset(vn[:, :, DH:DH + 1], 1.0)
                vb.append(vn)
            vns.append(vb)
            aos.append(ao_pool.tile([128, N], BF16, name=f"ao_{b}", tag=f"ao{b}"))
            bcs_all.append([None, None])
            ms_all.append(None)
            rstdj_all.append(None)

        def stats(b):
            """LN stats via matmul-with-ones; sum at row 0 (tile 0,0), sumsq at
            row 32 (tile 0,32) of ONE psum tile per ihalf. Produces ms_sb [2,N]
            bf16 ([mean; std] rows) and rstd_bc [128,1024] bf16 per ihalf."""
            scr = dscr_pool.tile([3, N], BF16, name=f"scr_{b}", tag=f"scr{b}")
            st = sm_pool.tile([128, 96], F32, name=f"st_{b}", tag="st128")
            for ihalf in range(2):
                # sum and sumsq chains in SEPARATE psum tiles (separate banks):
                # start=True clears has_written per-bank, so interleaved
                # accumulation chains must not share banks.
                sum_ps = ps_sc.tile([1, 1024], F32, name=f"sum_{b}_{ihalf}", tag="sc")
                sq_ps = ps_sc.tile([33, 1024], F32, name=f"ssq_{b}_{ihalf}", tag="sc")
                for kt in range(KT):
                    xt_t = xts[b][kt][ihalf]
                    xsq = sq_pool.tile([128, 1024], BF16, name=f"xsq_{b}_{ihalf}_{kt}", tag="sq")
                    nc.vector.tensor_mul(xsq, xt_t, xt_t)
                    for it2 in range(2):
                        s2 = slice(it2 * 512, (it2 + 1) * 512)
                        nc.tensor.matmul(
                            sum_ps[0:1, s2], ones_sb, xt_t[:, s2],
                            start=(kt == 0), stop=(kt == KT - 1),
                        )
                        nc.tensor.matmul(
                            sq_ps[32:33, s2], ones_sb, xsq[:, s2],
                            start=(kt == 0), stop=(kt == KT - 1),
                            tile_position=(0, 32),
                        )
                rows = sm_pool.tile([1, 2 * N], F32, name=f"rows_{b}_{ihalf}", tag="rows", bufs=1)
                nc.vector.tensor_copy(rows[0:1, 0:1024], sum_ps)
                nc.vector.tensor_copy(rows[0:1, 1024:2048], sq_ps[32:33, :])
                nc.sync.dma_start(out=st[:, ihalf * 8:(ihalf + 1) * 8], in_=rows[0:1, 0:1024])
                nc.sync.dma_start(out=st[:, 16 + ihalf * 8:16 + (ihalf + 1) * 8], in_=rows[0:1, 1024:2048])
            # st cols: 0:16 sum, 16:32 sumsq, 32:48 ex2, 48:64 mean, 64:80 std, 80:96 rstd
            ex2, mean = st[:, 32:48], st[:, 48:64]
            std, rstd = st[:, 64:80], st[:, 80:96]
            nc.vector.tensor_scalar_mul(mean, st[:, 0:16], 1.0 / D)
            nc.vector.tensor_scalar_mul(ex2, st[:, 16:32], 1.0 / D)
            nc.vector.tensor_mul(std, mean, mean)
            nc.vector.tensor_sub(std, std, ex2)  # mean^2 - E[x^2] = -var
            nc.scalar.activation(
                std, std, mybir.ActivationFunctionType.Sqrt,
                bias=eps_sb[:, 0:1], scale=-1.0,
            )  # sqrt(var + eps)
            nc.vector.reciprocal(rstd, std)
            stb = sm_pool.tile([128, 48], BF16, name=f"stb_{b}", tag="stb", bufs=2)
            nc.vector.tensor_copy(stb, st[:, 48:96])  # [mean | std | rstd] bf16
            for ihalf in range(2):
                c8 = slice(ihalf * 8, (ihalf + 1) * 8)
                isl = slice(ihalf * 1024, (ihalf + 1) * 1024)
                nc.sync.dma_start(out=scr[0:1, isl], in_=stb[:, 0:16][:, c8])
                nc.sync.dma_start(out=scr[1:2, isl], in_=stb[:, 16:32][:, c8])
                nc.sync.dma_start(out=scr[2:3, isl], in_=stb[:, 32:48][:, c8])
            ms_sb = ms_pool.tile([2, N], BF16, name=f"ms_{b}", tag="ms")
            nc.sync.dma_start(out=ms_sb, in_=scr[0:2, :])
            ms_all[b] = ms_sb
            # rstd per KEY token as [128, JC] columns: rstd_j[p, jc] = rstd(jc*128+p)
            rstd_jb = sm_pool.tile([128, JC], BF16, name=f"rstdjb_{b}", tag="rstdjb", bufs=2)
            for jc in range(JC):
                nc.sync.dma_start(
                    out=rstd_jb[:, jc:jc + 1], in_=scr[2:3, jc * 128:(jc + 1) * 128]
                )
            rstd_j = sm_pool.tile([128, JC], F32, name=f"rstdj_{b}", tag="rstdj", bufs=2)
            nc.vector.tensor_copy(rstd_j, rstd_jb)  # tensor_scalar needs f32 scalar
            rstdj_all[b] = rstd_j
            for ihalf in range(2):
                isl = slice(ihalf * 1024, (ihalf + 1) * 1024)
                rstd_bc = bc_pool.tile([128, 1024], BF16, name=f"rsbc_{b}_{ihalf}", tag="bc")
                nc.sync.dma_start(out=rstd_bc, in_=scr[2:3, isl].partition_broadcast(128))
                bcs_all[b][ihalf] = rstd_bc

        def qkv_group(b, cc, ihalf):
            """One q/k projection group: psum = W^T x + rank-1 LN terms, then
            DVE-evict (multiply by rstd broadcast) into qT/kT."""
            sb_dst = [qTs[b], kTs[b]]
            isl = slice(ihalf * 1024, (ihalf + 1) * 1024)
            pt = ps_acc.tile([128, 1024], F32, name=f"qp_{b}_{cc}_{ihalf}", tag="acc")
            for kt in range(KT):
                lhs = wqkv_sb[:, kt, cc * 128:(cc + 1) * 128]
                for it2 in range(2):
                    s2 = slice(it2 * 512, (it2 + 1) * 512)
                    bi = nc.tensor.matmul(
                        pt[:, s2], lhs, xts[b][kt][ihalf][:, s2],
                        start=(kt == 0), stop=False,
                    )
                    if it2 == 1:
                        bi.ins.ldweights = False
            lhs2 = wqkv2_sb[:, cc * 128:(cc + 1) * 128]
            for it2 in range(2):
                s2 = slice(it2 * 512, (it2 + 1) * 512)
                i2 = slice(ihalf * 1024 + it2 * 512, ihalf * 1024 + (it2 + 1) * 512)
                bi = nc.tensor.matmul(pt[:, s2], lhs2, ms_all[b][0:2, i2], start=False, stop=True)
                if it2 == 1:
                    bi.ins.ldweights = False
            nc.vector.tensor_mul(sb_dst[cc][:, isl], pt, bcs_all[b][ihalf])

        def vn_group(b, jc):
            """v in natural [j, ch] layout directly from PE: stationary = x
            d-chunk [128d, 128j], moving = wv [128d, 128ch]; out[j, ch]
            accumulates over d-chunks, + rank-1 LN terms with [mean; std] as
            the stationary [2, 128j]. Evict = per-partition rstd_j multiply."""
            ihalf, jloc = jc // 8, jc % 8
            jsl_t = slice(jloc * 128, (jloc + 1) * 128)   # within the x ihalf tile
            jsl = slice(jc * 128, (jc + 1) * 128)          # global (for ms_sb)
            vnp = ps_acc.tile([128, 128], F32, name=f"vnp_{b}_{jc}", tag="acc")
            for kt in range(KT):
                nc.tensor.matmul(
                    vnp, xts[b][kt][ihalf][:, jsl_t],
                    wqkv_sb[:, kt, 2 * 128:3 * 128],
                    start=(kt == 0), stop=False,
                )
            nc.tensor.matmul(
                vnp, ms_all[b][0:2, jsl], wqkv2_sb[:, 2 * 128:3 * 128],
                start=False, stop=True,
            )
            for hh in range(HL):
                nc.vector.tensor_scalar_mul(
                    vns[b][hh][:, jc, 0:DH], vnp[:, hh * DH:(hh + 1) * DH],
                    rstdj_all[b][:, jc:jc + 1],
                )

        aops = {}

        def attn_begin(b, ihalf):
            for hh in range(HL):
                aops[(b, hh)] = ps_acc.tile(
                    [DH + 1, 1024], F32, name=f"aop_{b}_{hh}_{ihalf}", tag="acc"
                )

        def attn_jc(b, ihalf, jc):
            """Scores (head-row-packed) -> exp -> *ea -> attn@v for both heads."""
            jsl = slice(jc * 128, (jc + 1) * 128)
            scs = [
                ps_sc.tile([128, 1024], F32, name=f"sc_{b}_{hh}_{ihalf}_{jc}", tag="sc")
                for hh in range(HL)
            ]
            # interleave h0/h1 per it2 so the two K=64 row-groups overlap on PE
            for it2 in range(2):
                s2 = slice(it2 * 512, (it2 + 1) * 512)
                i2 = slice(ihalf * 1024 + it2 * 512, ihalf * 1024 + (it2 + 1) * 512)
                for hh in range(HL):
                    hsl = slice(hh * DH, (hh + 1) * DH)
                    bi = nc.tensor.matmul(
                        scs[hh][:, s2], kTs[b][hsl, jsl], qTs[b][hsl, i2],
                        start=True, stop=True,
                    )
                    if it2 == 1:
                        bi.ins.ldweights = False
            for hh in range(HL):
                ea_t = ea_pool.tile([128, 1024], BF16, name=f"ea_{b}_{hh}_{ihalf}_{jc}", tag="ea")
                nc.gpsimd.dma_start(out=ea_t, in_=ea_d[hh, ihalf, jc])
                ate = ate_pool.tile([128, 1024], BF16, name=f"ate_{b}_{hh}_{ihalf}_{jc}", tag="ate")
                nc.scalar.activation(ate, scs[hh], mybir.ActivationFunctionType.Exp)
                at_t = at_pool.tile([128, 1024], BF16, name=f"at_{b}_{hh}_{ihalf}_{jc}", tag="at")
                nc.vector.tensor_mul(at_t, ate, ea_t)
                for it2 in range(2):
                    s2 = slice(it2 * 512, (it2 + 1) * 512)
                    bi = nc.tensor.matmul(
                        aops[(b, hh)][:, s2], vns[b][hh][:, jc, :], at_t[:, s2],
                        start=(jc == 0), stop=(jc == JC - 1),
                    )
                    if it2 == 1:
                        bi.ins.ldweights = False

        scr3 = dscr_pool.tile([16, 1024], F32, name="scr3", tag="scr3")
        scr4 = dscr_pool.tile([16, 1024], F32, name="scr4", tag="scr4")

        def attn_end(b, ihalf):
            """Evict raw attn output (frees PSUM fast); reciprocal of the
            denominators via [128,8] DMA reshape; normalize off critical path."""
            isl = slice(ihalf * 1024, (ihalf + 1) * 1024)
            for hh in range(HL):
                hsl = slice(hh * DH, (hh + 1) * DH)
                r = (hh * 2 + ihalf) * 2 + b
                aop = aops.pop((b, hh))
                ao_raw = aor_pool.tile([DH + 1, 1024], F32, name=f"aor_{r}", tag="aor")
                nc.vector.tensor_copy(ao_raw, aop)
                nc.sync.dma_start(out=scr3[r:r + 1, :], in_=ao_raw[DH:DH + 1, :])
                r128 = sm_pool.tile([128, 8], F32, name=f"r128_{r}", tag="r128", bufs=3)
                nc.sync.dma_start(out=r128, in_=scr3[r:r + 1, :])
                nc.vector.reciprocal(r128, r128)
                nc.sync.dma_start(out=scr4[r:r + 1, :], in_=r128)
                rr_bc = rrbc_pool.tile([DH, 1024], F32, name=f"rrbc_{r}", tag="rrbc")
                nc.sync.dma_start(
                    out=rr_bc, in_=scr4[r:r + 1, :].partition_broadcast(DH)
                )
                nc.vector.tensor_mul(aos[b][hsl, isl], ao_raw[0:DH, :], rr_bc)

        def outproj(b, ihalf):
            isl = slice(ihalf * 1024, (ihalf + 1) * 1024)
            for ec in range(8):
                lhs = wout_sb[:, ec * 128:(ec + 1) * 128]
                opp = ps_acc.tile([128, 1024], F32, name=f"op_{b}_{ec}_{ihalf}", tag="acc")
                for it2 in range(2):
                    s2 = slice(it2 * 512, (it2 + 1) * 512)
                    i2 = slice(ihalf * 1024 + it2 * 512, ihalf * 1024 + (it2 + 1) * 512)
                    bi = nc.tensor.matmul(opp[:, s2], lhs, aos[b][:, i2], start=True, stop=True)
                    if it2 == 1:
                        bi.ins.ldweights = False
                ob = ob_pool.tile([128, 1024], BF16, name=f"ob_{b}_{ec}_{ihalf}", tag="ob")
                if ec % 2 == 0:
                    nc.scalar.copy(ob, opp)
                else:
                    nc.vector.tensor_copy(ob, opp)
                nc.sync.dma_start(out=out_d[b, ec * 128:(ec + 1) * 128, isl], in_=ob)

        # ================= emission schedule =================
        stats(0)
        qkv_group(0, 1, 0)   # k-ih0
        qkv_group(0, 0, 0)   # q-ih0
        for jc in range(8):
            vn_group(0, jc)
        qkv_group(0, 1, 1)   # k-ih1
        qkv_group(0, 0, 1)   # q-ih1
        for jc in range(8, JC):
            vn_group(0, jc)
        stats(1)

        attn_begin(0, 0)
        for jc in range(JC):
            attn_jc(0, 0, jc)
        attn_end(0, 0)

        qkv_group(1, 1, 0)   # k-ih0
        qkv_group(1, 0, 0)   # q-ih0
        for jc in range(4):
            vn_group(1, jc)

        attn_begin(0, 1)
        for jc in range(JC):
            attn_jc(0, 1, jc)
        attn_end(0, 1)

        for jc in range(4, 8):
            vn_group(1, jc)
        qkv_group(1, 1, 1)   # k-ih1
        for jc in range(8, JC):
            vn_group(1, jc)

        attn_begin(1, 0)
        for jc in range(JC):
            attn_jc(1, 0, jc)
        attn_end(1, 0)

        qkv_group(1, 0, 1)   # q-ih1
        outproj(0, 0)
        outproj(0, 1)

        attn_begin(1, 1)
        for jc in range(JC):
            attn_jc(1, 1, jc)
        attn_end(1, 1)

        outproj(1, 0)
        outproj(1, 1)
    nc.compile()
    return nc


def make_in_maps(x, alibi_bias, ln_gamma, ln_beta, w_qkv, w_out):
    """Host-side sharding / layout prep. Returns list of 8 per-core input dicts."""
    x = np.asarray(x, np.float32)
    alibi_bias = np.asarray(alibi_bias, np.float32)
    ln_gamma = np.asarray(ln_gamma, np.float32)
    ln_beta = np.asarray(ln_beta, np.float32)
    w_qkv = np.asarray(w_qkv, np.float32)
    w_out = np.asarray(w_out, np.float32)
    BF = ml_dtypes.bfloat16

    xt = np.ascontiguousarray(x.transpose(0, 2, 1)).astype(BF)  # [B, D, N]
    # fold ln_gamma into w_qkv rows; fold attention scale into the q columns
    w_eff = w_qkv * ln_gamma[:, None]
    qkvb_full = ln_beta @ w_qkv  # [3*H*DH]
    in_maps = []
    for c in range(NCORES):
        csl = slice(c * CL, (c + 1) * CL)
        wq = w_eff[:, 0:H * DH][:, csl] * SCALE
        wk = w_eff[:, H * DH:2 * H * DH][:, csl]
        wv = w_eff[:, 2 * H * DH:3 * H * DH][:, csl]
        wqkv_c = np.ascontiguousarray(np.concatenate([wq, wk, wv], axis=1)).astype(BF)
        nwsum_c = -wqkv_c.astype(np.float64).sum(axis=0).astype(np.float32)
        qb = qkvb_full.reshape(3, H * DH)[:, csl].copy()
        qb[0] *= SCALE
        qkvb_c = qb.reshape(-1)
        wqkv2_c = np.ascontiguousarray(
            np.stack([nwsum_c, qkvb_c], axis=0)
        ).astype(BF)  # [2, 3CL]: [nwsum; qkvb]
        # ea = exp(alibi^T) tiled [hh, ihalf, jc, 128j, 1024i], contiguous tiles
        al_t = alibi_bias[0, c * HL:(c + 1) * HL].transpose(0, 2, 1)  # [hh, j, i]
        ea = np.exp(al_t)
        ea_c = np.ascontiguousarray(
            ea.reshape(HL, JC, 128, 2, 1024).transpose(0, 3, 1, 2, 4)
        ).astype(BF)
        wout_c = np.ascontiguousarray(w_out[csl, :]).astype(BF)
        in_maps.append({
            "xt": xt,
            "ea": ea_c,
            "wqkv": wqkv_c,
            "wqkv2": wqkv2_c,
            "wout": wout_c,
            "ones": np.ones((128, 1), BF),
        })
    return in_maps


def kernel(x, alibi_bias, mask, ln_gamma, ln_beta, w_qkv, w_out, _trace=False):
    global _CACHED_NC
    mask = np.asarray(mask)
    assert mask.all(), "kernel assumes an all-True mask"
    if _CACHED_NC is None:
        _CACHED_NC = build_nc()
    nc = _CACHED_NC
    in_maps = make_in_maps(x, alibi_bias, ln_gamma, ln_beta, w_qkv, w_out)
    res = run_bass_kernel_spmd(nc, in_maps, core_ids=list(range(NCORES)), trace=_trace)
    out_t = np.zeros((B, D, N), np.float32)
    for c in range(NCORES):
        out_t += res.results[c]["out"].astype(np.float32)
    out = np.ascontiguousarray(out_t.transpose(0, 2, 1))
    if _trace:
        return out, res
    return out
